# revision 14
# baseline (speedup 1.0000x reference)
"""PosetBlock Trainium2 kernel: banded message-passing attention block.

Self-contained: takes full inputs, shards the token axis across 8 NeuronCores
(halo recompute), runs a Bass/Tile kernel via run_bass_kernel_spmd, and
reassembles the full output.
"""

import numpy as np

# Problem geometry (hardcoded per contract).
B, T, D = 4, 2048, 1024
H, DH, KP, WINDOW = 16, 64, 16, 128
TAU, ITERS = 0.07, 3
PEXP = 1.0 / TAU  # 14.285714285714286

NCORES = 8
OWN = T // NCORES          # 256 tokens owned per core
HALO = 128                 # >= 4*16 recursion depth + margins
WIN = OWN + HALO           # 384-token window, = 3 dst tiles of 128
DTILE = 128
NT = WIN // DTILE          # 3
NJD = NT * DTILE           # 384 hi-score cols per head
LOC = H * 2 * KP           # 512 lo-score cols (2 lo tiles x 16 dst per head)
NEG = -1.0e30

_CACHE = {}


def _build_program():
    import concourse.bacc as bacc
    import concourse.mybir as mybir
    import concourse.tile as tile

    f32 = mybir.dt.float32
    Alu = mybir.AluOpType
    Act = mybir.ActivationFunctionType
    Ax = mybir.AxisListType

    nc = bacc.Bacc("TRN2", target_bir_lowering=False, debug=False,
                   num_devices=NCORES)

    xw = nc.dram_tensor("xw", [B, WIN, D], f32, kind="ExternalInput")
    wq = nc.dram_tensor("wq", [D, D], f32, kind="ExternalInput")
    wk = nc.dram_tensor("wk", [D, D], f32, kind="ExternalInput")
    wv = nc.dram_tensor("wv", [D, D], f32, kind="ExternalInput")
    wo = nc.dram_tensor("wo", [D, D], f32, kind="ExternalInput")
    bqd = nc.dram_tensor("bq", [1, D], f32, kind="ExternalInput")
    bkd = nc.dram_tensor("bk", [1, D], f32, kind="ExternalInput")
    bvd = nc.dram_tensor("bv", [1, D], f32, kind="ExternalInput")
    bias_hi_d = nc.dram_tensor("bias_hi", [128, NJD], f32,
                               kind="ExternalInput")
    bias_lo_d = nc.dram_tensor("bias_lo", [KP, 2 * KP], f32,
                               kind="ExternalInput")
    identd = nc.dram_tensor("ident", [128, 128], f32, kind="ExternalInput")
    outd = nc.dram_tensor("out", [B, OWN, D], f32, kind="ExternalOutput")

    from contextlib import ExitStack

    with ExitStack() as ctx:
        tc = ctx.enter_context(tile.TileContext(nc))
        cpool = ctx.enter_context(tc.tile_pool(name="const", bufs=1))
        px = ctx.enter_context(tc.tile_pool(name="px", bufs=3))
        scr = ctx.enter_context(tc.tile_pool(name="scr", bufs=1))
        stat = ctx.enter_context(tc.tile_pool(name="stat", bufs=4))
        pyT = ctx.enter_context(tc.tile_pool(name="yT", bufs=1))
        pqT = ctx.enter_context(tc.tile_pool(name="qT", bufs=1))
        pkT = ctx.enter_context(tc.tile_pool(name="kT", bufs=1))
        pv = ctx.enter_context(tc.tile_pool(name="vsb", bufs=1))
        pw = ctx.enter_context(tc.tile_pool(name="wch", bufs=8))
        pchain = ctx.enter_context(tc.tile_pool(name="chain", bufs=1))
        psv = ctx.enter_context(tc.tile_pool(name="sv", bufs=10))
        phT = ctx.enter_context(tc.tile_pool(name="hT", bufs=1))
        phsb = ctx.enter_context(tc.tile_pool(name="hsb", bufs=2))
        pout = ctx.enter_context(tc.tile_pool(name="outp", bufs=2))
        pps = ctx.enter_context(tc.tile_pool(name="pps", bufs=2, space="PSUM"))
        ppL = ctx.enter_context(tc.tile_pool(name="ppL", bufs=3, space="PSUM"))
        ppag = ctx.enter_context(tc.tile_pool(name="ppag", bufs=3,
                                              space="PSUM"))
        if True:
            # constants
            ident = cpool.tile([128, 128], f32, tag="ident")
            nc.sync.dma_start(ident[:], identd[:])
            bias_hi = cpool.tile([128, NJD], f32, tag="bias_hi")
            nc.sync.dma_start(bias_hi[:], bias_hi_d[:])
            bias_lo = cpool.tile([KP, 2 * KP], f32, tag="bias_lo")
            nc.sync.dma_start(bias_lo[:], bias_lo_d[:])
            ones = cpool.tile([1, 512], f32, tag="ones")
            nc.gpsimd.memset(ones[:], 1.0)
            bq_sb = cpool.tile([1, D], f32, tag="bq")
            nc.sync.dma_start(bq_sb[:], bqd[:])
            bk_sb = cpool.tile([1, D], f32, tag="bk")
            nc.sync.dma_start(bk_sb[:], bkd[:])
            bv_sb = cpool.tile([1, D], f32, tag="bv")
            nc.sync.dma_start(bv_sb[:], bvd[:])
            c_eps = cpool.tile([128, 1], f32, tag="c_eps")
            nc.gpsimd.memset(c_eps[:], 1e-5)
            c_one = cpool.tile([128, 1], f32, tag="c_one")
            nc.gpsimd.memset(c_one[:], 1.0)
            # zero-padded lo weights: rows [112:128) filled per batch by DMA
            a_lop = []
            for i in range(2):
                t = cpool.tile([128, LOC], f32, tag=f"alop{i}",
                               name=f"alop{i}")
                nc.gpsimd.memset(t[0:112, :], 0.0)
                a_lop.append(t)

            for b in range(B):
                # ---- LayerNorm + transpose: y_T[dc] = [128 d, 384 tok] ----
                y_T = [pyT.tile([128, WIN], f32, tag=f"yT{dc}",
                                name=f"yT{dc}") for dc in range(8)]
                for it in range(NT):
                    t0 = it * 128
                    xt = px.tile([128, D], f32, tag="x")
                    nc.sync.dma_start(xt[:], xw[b, t0:t0 + 128, :])
                    s1 = stat.tile([128, 1], f32, tag="s1")
                    nc.vector.tensor_reduce(s1[:], xt[:], op=Alu.add,
                                            axis=Ax.X)
                    sq = scr.tile([128, D], f32, tag="sq")
                    ss = stat.tile([128, 1], f32, tag="ss")
                    nc.scalar.activation(sq[:], xt[:], Act.Square,
                                         accum_out=ss[:])
                    mu = stat.tile([128, 1], f32, tag="mu")
                    nc.vector.tensor_scalar(mu[:], s1[:], 1.0 / D, None,
                                            Alu.mult)
                    mu2 = stat.tile([128, 1], f32, tag="mu2")
                    nc.vector.tensor_tensor(mu2[:], mu[:], mu[:], Alu.mult)
                    var = stat.tile([128, 1], f32, tag="var")
                    nc.vector.tensor_scalar(var[:], ss[:], 1.0 / D, None,
                                            Alu.mult)
                    nc.vector.tensor_tensor(var[:], var[:], mu2[:],
                                            Alu.subtract)
                    # rstd = exp(-0.5 * ln(var + 1e-5))
                    lv = stat.tile([128, 1], f32, tag="lv")
                    nc.scalar.activation(lv[:], var[:], Act.Ln, bias=c_eps[:])
                    rstd = stat.tile([128, 1], f32, tag="rstd")
                    nc.scalar.activation(rstd[:], lv[:], Act.Exp, scale=-0.5)
                    yt = scr.tile([128, D], f32, tag="yt")
                    nc.vector.tensor_scalar(yt[:], xt[:], mu[:], rstd[:],
                                            Alu.subtract, Alu.mult)
                    for dc in range(8):
                        pst = pps.tile([128, 128], f32, tag="tp")
                        nc.tensor.transpose(pst[:],
                                            yt[:, dc * 128:(dc + 1) * 128],
                                            ident[:])
                        nc.scalar.copy(y_T[dc][:, t0:t0 + 128], pst[:])

                # ---- projections q_T, k_T: [128 qdim, 384 tok] x 8 ----
                def proj_qk(wdram, brow, pool, name):
                    wsb = [pw.tile([128, D], f32, tag="wch",
                                   name=f"w{i}") for i in range(8)]
                    for kc in range(8):
                        nc.sync.dma_start(wsb[kc][:],
                                          wdram[kc * 128:(kc + 1) * 128, :])
                    outs = []
                    for ot in range(8):
                        ps = ppL.tile([128, WIN], f32, tag="Lps")
                        for kc in range(8):
                            nc.tensor.matmul(
                                ps[:], wsb[kc][:, ot * 128:(ot + 1) * 128],
                                y_T[kc][:], start=(kc == 0), stop=False)
                        nc.tensor.matmul(ps[:],
                                         brow[0:1, ot * 128:(ot + 1) * 128],
                                         ones[0:1, :WIN], start=False,
                                         stop=True)
                        ot_sb = pool.tile([128, WIN], f32,
                                          tag=f"{name}{ot}",
                                          name=f"{name}{ot}")
                        nc.scalar.copy(ot_sb[:], ps[:])
                        outs.append(ot_sb)
                    return outs

                q_T = proj_qk(wq, bq_sb, pqT, "qT")
                k_T = proj_qk(wk, bk_sb, pkT, "kT")

                # ---- v in S-tile layout: v_sb[i] = [128 tok, 16*(64+1)] ----
                wsb = [pw.tile([128, D], f32, tag="wch", name=f"wv{i}")
                       for i in range(8)]
                for kc in range(8):
                    nc.sync.dma_start(wsb[kc][:],
                                      wv[kc * 128:(kc + 1) * 128, :])
                v_sb = []
                for i in range(NT):
                    vj = pv.tile([128, H * 65], f32, tag=f"v{i}",
                                 name=f"v{i}")
                    r0 = i * DTILE
                    for half in range(2):
                        ps = ppL.tile([128, 512], f32, tag="Lps")
                        for kc in range(8):
                            nc.tensor.matmul(
                                ps[:], y_T[kc][:, r0:r0 + 128],
                                wsb[kc][:, half * 512:(half + 1) * 512],
                                start=(kc == 0), stop=False)
                        nc.tensor.matmul(
                            ps[:], ones[0:1, 0:128],
                            bv_sb[0:1, half * 512:(half + 1) * 512],
                            start=False, stop=True)
                        vjv = vj[:].rearrange("p (h c) -> p h c", c=65)
                        psv_ = ps[:].rearrange("p (h c) -> p h c", c=64)
                        nc.vector.tensor_copy(
                            vjv[:, half * 8:(half + 1) * 8, 0:64], psv_[:])
                    vjv = vj[:].rearrange("p (h c) -> p h c", c=65)
                    nc.gpsimd.memset(vjv[:, :, 64:65], 1.0)
                    v_sb.append(vj)

                # ---- scores ----
                m_all = pchain.tile([128, H * NJD + LOC], f32, tag="mall")
                a_all = pchain.tile([128, H * NJD + LOC], f32, tag="aall")
                nc.gpsimd.memset(m_all[:, H * NJD:H * NJD + LOC], 0.0)
                for h in range(H):
                    hh, hp = h // 2, (h % 2) * 64
                    ps = ppL.tile([128, NJD + 2 * KP], f32, tag="Lps")
                    for i in range(NT):
                        r0 = i * DTILE
                        nc.tensor.matmul(
                            ps[:, r0:r0 + 128],
                            k_T[hh][hp:hp + 64, r0:r0 + 128],
                            q_T[hh][hp:hp + 64, r0:r0 + 128],
                            start=True, stop=True)
                        if i >= 1:
                            nc.tensor.matmul(
                                ps[0:KP, NJD + (i - 1) * KP:
                                   NJD + i * KP],
                                k_T[hh][hp:hp + 64, r0 - KP:r0],
                                q_T[hh][hp:hp + 64, r0:r0 + KP],
                                start=True, stop=True)
                    nc.vector.tensor_tensor(
                        m_all[:, h * NJD:(h + 1) * NJD], ps[:, 0:NJD],
                        bias_hi[:], Alu.add)
                    nc.vector.tensor_tensor(
                        m_all[0:KP, H * NJD + h * 2 * KP:
                              H * NJD + (h + 1) * 2 * KP],
                        ps[0:KP, NJD:NJD + 2 * KP], bias_lo[:], Alu.add)
                # a = exp(-PEXP * ln(1 + min(exp(-0.125*m), 1e18)))
                nc.scalar.activation(a_all[:], m_all[:], Act.Exp, scale=-0.125)
                nc.vector.tensor_scalar(a_all[:], a_all[:], 1e18, None,
                                        Alu.min)
                nc.scalar.activation(m_all[:], a_all[:], Act.Ln, bias=c_one[:])
                nc.scalar.activation(a_all[:], m_all[:], Act.Exp, scale=-PEXP)
                # scatter lo weights into zero-padded lhsT rows [112:128)
                nc.sync.dma_start(a_lop[b % 2][112:128, :],
                                  a_all[0:KP, H * NJD:H * NJD + LOC])

                # ---- aggregation (4 passes) + h + h_T ----
                h_T = [phT.tile([128, OWN], f32, tag=f"hT{hc}",
                                name=f"hT{hc}") for hc in range(8)]
                for h in range(H):
                    hh, hp = h // 2, (h % 2) * 64
                    vsl = [v_sb[i][:, h * 65:h * 65 + 65] for i in range(NT)]
                    S = {0: vsl}
                    if h % 2 == 0:
                        hsp = [phsb.tile([128, 128], f32, tag=f"hsp{i}",
                                         name=f"hsp{i}") for i in range(2)]
                    for p in range(1, 5):
                        if p <= 3:
                            S[p] = [psv.tile([128, 65], f32, tag="sv",
                                             name=f"s{p}{jj}")
                                    for jj in range(NT)]
                        for i in range(NT):
                            if p == 4 and i == 0:
                                continue
                            pa = ppag.tile([128, 65], f32, tag="ag")
                            nc.tensor.matmul(
                                pa[:],
                                a_all[:, h * NJD + i * DTILE:
                                      h * NJD + (i + 1) * DTILE],
                                S[p - 1][i][:], start=True, stop=True)
                            if i >= 1:
                                nc.tensor.matmul(
                                    pa[0:KP, :],
                                    a_lop[b % 2][:, h * 2 * KP +
                                                 (i - 1) * KP:
                                                 h * 2 * KP + i * KP],
                                    S[p - 1][i - 1][:], start=False,
                                    stop=True, skip_group_check=True)
                            if p <= 3:
                                nc.vector.tensor_tensor(S[p][i][:], pa[:],
                                                        vsl[i], Alu.add)
                            else:
                                # h = SB / max(SZ, 1e-9)
                                zm = stat.tile([128, 1], f32, tag="zm")
                                nc.vector.tensor_scalar(zm[:], pa[:, 64:65],
                                                        1e-9, None, Alu.max)
                                zr = stat.tile([128, 1], f32, tag="zr")
                                nc.vector.reciprocal(zr[:], zm[:])
                                nc.vector.tensor_scalar(
                                    hsp[i - 1][:, hp:hp + 64], pa[:, 0:64],
                                    zr[:], None, Alu.mult)
                    if h % 2 == 1:
                        for i in range(2):
                            pt = pps.tile([128, 128], f32, tag="tp")
                            nc.tensor.transpose(pt[:], hsp[i][:], ident[:])
                            nc.scalar.copy(h_T[hh][:, i * 128:(i + 1) * 128],
                                           pt[:])

                # ---- output projection + residual ----
                wob = [pw.tile([128, D], f32, tag="wch", name=f"wo{i}")
                       for i in range(8)]
                for kc in range(8):
                    nc.sync.dma_start(wob[kc][:],
                                      wo[kc * 128:(kc + 1) * 128, :])
                for ot2 in range(2):
                    xr = px.tile([128, D], f32, tag="x")
                    nc.sync.dma_start(
                        xr[:], xw[b, 128 + ot2 * 128:128 + (ot2 + 1) * 128,
                                  :])
                    op = pout.tile([128, D], f32, tag="outp")
                    for half in range(2):
                        ps = ppL.tile([128, 512], f32, tag="Lps")
                        for kc in range(8):
                            nc.tensor.matmul(
                                ps[:], h_T[kc][:, ot2 * 128:(ot2 + 1) * 128],
                                wob[kc][:, half * 512:(half + 1) * 512],
                                start=(kc == 0), stop=(kc == 7))
                        nc.vector.tensor_tensor(
                            op[:, half * 512:(half + 1) * 512], ps[:],
                            xr[:, half * 512:(half + 1) * 512], Alu.add)
                    nc.sync.dma_start(outd[b, ot2 * 128:(ot2 + 1) * 128, :],
                                      op[:])

    nc.compile()
    return nc


def _banded_ok(src_idx, dst_idx, delta):
    t = np.arange(T)
    counts = np.minimum(t, KP)
    E = int(counts.sum())
    if len(src_idx) != E:
        return False
    dst_e = np.repeat(t, counts)
    starts = np.repeat(np.maximum(t - KP, 0), counts)
    within = np.arange(E) - np.repeat(np.cumsum(counts) - counts, counts)
    src_e = starts + within
    return (np.array_equal(np.asarray(dst_idx), dst_e)
            and np.array_equal(np.asarray(src_idx), src_e)
            and np.array_equal(np.asarray(delta), dst_e - src_e))


def _make_in_maps(x, wq_t, wk_t, wv_t, wo_t, bq, bk, bv, rb, ident):
    in_maps = []
    kk = np.arange(128)[:, None]
    mm = np.arange(DTILE)[None, :]
    kl = np.arange(KP)[:, None]
    ml = np.arange(KP)[None, :]
    for c in range(NCORES):
        wstart = c * OWN - HALO
        xwc = np.zeros((B, WIN, D), np.float32)
        v0 = max(wstart, 0)
        xwc[:, v0 - wstart:, :] = x[:, v0:wstart + WIN, :]
        bias_hi = np.empty((128, NJD), np.float32)
        for i in range(NT):
            s_tok = wstart + i * DTILE + kk
            d_tok = wstart + i * DTILE + mm
            de = d_tok - s_tok
            valid = (de >= 1) & (de <= KP) & (s_tok >= 0) & (d_tok >= 0)
            vals = 8.0 * rb[np.clip(de, 0, WINDOW)]
            bias_hi[:, i * DTILE:(i + 1) * DTILE] = np.where(valid, vals, NEG)
        bias_lo = np.empty((KP, 2 * KP), np.float32)
        for i in (1, 2):
            s_tok = wstart + i * DTILE - KP + kl
            d_tok = wstart + i * DTILE + ml
            de = d_tok - s_tok
            valid = (de >= 1) & (de <= KP) & (s_tok >= 0) & (d_tok >= 0)
            vals = 8.0 * rb[np.clip(de, 0, WINDOW)]
            bias_lo[:, (i - 1) * KP:i * KP] = np.where(valid, vals, NEG)
        in_maps.append({
            "xw": xwc, "wq": wq_t, "wk": wk_t, "wv": wv_t, "wo": wo_t,
            "bq": bq, "bk": bk, "bv": bv, "bias_hi": bias_hi,
            "bias_lo": bias_lo, "ident": ident,
        })
    return in_maps


def _host_prep(x, ln_g, ln_b, Wq, Wk, Wv, Wo, rel_bias):
    x = np.ascontiguousarray(np.asarray(x, np.float32))
    ln_g = np.asarray(ln_g, np.float32)
    ln_b = np.asarray(ln_b, np.float32)
    Wq = np.asarray(Wq, np.float32)
    Wk = np.asarray(Wk, np.float32)
    Wv = np.asarray(Wv, np.float32)
    Wo = np.asarray(Wo, np.float32)
    rel_bias = np.asarray(rel_bias, np.float32)
    wq_t = np.ascontiguousarray((Wq * ln_g[None, :]).T)
    wk_t = np.ascontiguousarray((Wk * ln_g[None, :]).T)
    wv_t = np.ascontiguousarray((Wv * ln_g[None, :]).T)
    wo_t = np.ascontiguousarray(Wo.T)
    bq = (Wq @ ln_b)[None, :].astype(np.float32)
    bk = (Wk @ ln_b)[None, :].astype(np.float32)
    bv = (Wv @ ln_b)[None, :].astype(np.float32)
    ident = np.eye(128, dtype=np.float32)
    rb = rel_bias[:, 0]
    return _make_in_maps(x, wq_t, wk_t, wv_t, wo_t, bq, bk, bv, rb, ident)


def _reference_numpy(x, ln_g, ln_b, Wq, Wk, Wv, Wo, rel_bias, src_idx,
                     dst_idx, delta):
    # General fallback (any edge list); matches reference.py math.
    x = np.asarray(x, np.float32)
    mu = x.mean(-1, keepdims=True)
    var = x.var(-1, keepdims=True)
    x_ln = (x - mu) / np.sqrt(var + 1e-5) * ln_g + ln_b
    Bz = B * H

    def proj(W):
        return (x_ln @ W.T).reshape(B, T, H, DH).transpose(0, 2, 1, 3) \
            .reshape(Bz, T, DH)

    q, k, v = proj(np.asarray(Wq)), proj(np.asarray(Wk)), proj(np.asarray(Wv))
    logits = np.einsum('zed,zed->ze', q[:, dst_idx], k[:, src_idx]) / np.sqrt(
        np.float32(DH))
    dd = np.clip(delta, 0, WINDOW)
    logits = logits + np.asarray(rel_bias)[dd, 0][None, :]
    I = np.clip(1.0 / (1.0 + np.exp(-logits)), 1e-6, 1 - 1e-6)
    a = I ** np.float32(PEXP)

    def agg(Bs, Z):
        SB = np.zeros((Bz, T, DH), np.float32)
        SZ = np.zeros((Bz, T), np.float32)
        np.add.at(SB, (slice(None), dst_idx), a[:, :, None] * Bs[:, src_idx])
        np.add.at(SZ, (slice(None), dst_idx), a * Z[:, src_idx])
        return SB, SZ

    Bs, Z = v, np.ones((Bz, T), np.float32)
    for _ in range(ITERS):
        SB, SZ = agg(Bs, Z)
        Bs, Z = v + SB, 1.0 + SZ
    SB, SZ = agg(Bs, Z)
    hh = SB / np.maximum(SZ, 1e-9)[..., None]
    hh = hh.reshape(B, H, T, DH).transpose(0, 2, 1, 3).reshape(B, T, D)
    return x + hh @ np.asarray(Wo).T


def kernel(x, ln_g, ln_b, Wq, Wk, Wv, Wo, rel_bias, src_idx, dst_idx, delta):
    src_idx = np.asarray(src_idx)
    dst_idx = np.asarray(dst_idx)
    delta = np.asarray(delta)

    if not _banded_ok(src_idx, dst_idx, delta):
        return _reference_numpy(x, ln_g, ln_b, Wq, Wk, Wv, Wo, rel_bias,
                                src_idx, dst_idx, delta).astype(np.float32)

    from concourse.bass_utils import run_bass_kernel_spmd

    if "nc" not in _CACHE:
        _CACHE["nc"] = _build_program()
    nc = _CACHE["nc"]

    in_maps = _host_prep(x, ln_g, ln_b, Wq, Wk, Wv, Wo, rel_bias)
    _CACHE["last_in_maps"] = in_maps

    res = run_bass_kernel_spmd(nc, in_maps, list(range(NCORES)))
    out = np.empty((B, T, D), np.float32)
    for c in range(NCORES):
        out[:, c * OWN:(c + 1) * OWN, :] = res.results[c]["out"]
    return out
